# revision 31
# baseline (speedup 1.0000x reference)
"""3-layer GAT on 8 Trainium2 NeuronCores (Bass/Tile).

Sharding: 2D graph partition. Pair q = cores {2q, 2q+1} aggregates the dst
nodes of strips [q*6250,(q+1)*6250) and [25000+q*6250, 25000+(q+1)*6250);
even cores take edges with src < 25000, odd cores the rest. Within each
pair, dst rows are re-packed into 128-row blocks balancing per-core edge
counts, and blocks are ordered in 8 interleaved (even, odd) segments so the
pairwise ReduceScatter, the divide/bias/ELU post-processing, the next
layer's projection, and most of the quad AllGather all run *during* the
edge phase of the current layer, gated per segment.

Per layer: each core projects its own rows (feat|el|er via an augmented
weight matrix) into bf16 gather-table rows (feat|el_hi|el_lo), AllGathers
the table across its src-half quad (in 3 parts) and tiny er_hi/er_lo rows
across its pair, then streams edges (grouped per dst block, padded per
128-edge tile) through: dma_gather of source rows; per-edge er delivered by
building the transposed dst-slot one-hot with a partition-broadcast DMA +
is_equal and multiplying it with the block's er rows (one small matmul per
tile — no second dma_gather); edge softmax without segment-max; messages
accumulated per dst block by one-hot matmuls into PSUM. Gather tables are
double-buffered across layers so the next layer's AllGather overlaps this
layer's gathers. Host-side: inputs are uploaded once and cached on device
across calls; indices ship deduplicated ([16,*]) and features in bf16.
"""

import numpy as np
import ml_dtypes

N = 50000
E = 800000
F = 128                  # input feats and hidden width (4 heads x 32)
H = 4
D = 32
NEG = 0.2
NCORE = 8
NPC = 6250               # nodes owned per core
OWN = 6272               # 49*128, padded own rows
OWNBLK = 49
PAIR = 12544             # 98*128 dst slots per pair
NBLK = 98
HALF = 25088             # 4*OWN rows per src-half table
TROWS = 25216            # HALF + 128 (dummy row at HALF)
DUMMY = HALF
TCOLS = 256              # bf16 cols: feat(128) | el_hi(4) | el_lo(4) | pad
CHUNK = 32               # tiles per dma_gather call
GROUP = 16               # tiles per vector-op batch
EPS = 1e-30
CAP = 1024               # target edges per (core, block) = 8 tiles

# segment structure: local (own) blocks per RS segment, and AllGather parts
# in units of segments
ECUM = [0, 7, 13, 19, 25, 31, 37, 43, 49]      # 8 segments of own blocks
NSEG = 8
AGPART = [(0, 25), (25, 43), (43, 49)]          # own-block ranges per AG part
AGOFF = [0, 12800, 22016]                       # table row offset per part

_cache = {}


def _pack(d0, d1):
    """Assign len(d0) rows to OWNBLK blocks of <=128 rows, minimizing the
    number of 128-edge tiles each block needs on either core: greedy with
    per-block caps (unavoidable overflow concentrated in the first K
    blocks), then swap repair, then heaviest-first renumbering so leftover
    9-tile blocks align across pairs (T_b is a cross-pair max)."""
    n = len(d0)
    tot = int(max(d0.sum(), d1.sum()))
    K = max(0, -(-(tot - OWNBLK * CAP) // 128))
    caps = np.full(OWNBLK, CAP, np.int64)
    caps[:K] += 128
    order = np.argsort(-(d0 + d1), kind="stable")
    L0 = np.zeros(OWNBLK, np.int64)
    L1 = np.zeros(OWNBLK, np.int64)
    cnt = np.zeros(OWNBLK, np.int64)
    blk = np.empty(n, np.int64)
    BIG = 1 << 40
    for r in order:
        load = np.maximum(L0 + d0[r], L1 + d1[r])
        bad = ((cnt >= 128) | (L0 + d0[r] > caps) | (L1 + d1[r] > caps)) * BIG
        b = int(np.argmin(load + bad))
        if load[b] + bad[b] >= BIG:
            b = int(np.argmin(load + (cnt >= 128) * BIG))
        blk[r] = b
        L0[b] += d0[r]
        L1[b] += d1[r]
        cnt[b] += 1

    # swap repair: fix blocks exceeding caps by exchanging a heavy row for a
    # lighter row from a block with slack
    stuck = set()
    for _ in range(3000):
        over = np.maximum(L0 - caps, L1 - caps)
        viol = [b for b in np.argsort(-over) if over[b] > 0 and b not in stuck]
        if not viol:
            break
        b = int(viol[0])
        r0 = max(0, int(L0[b] - caps[b]))
        r1 = max(0, int(L1[b] - caps[b]))
        rows_b = np.where(blk == b)[0]
        pref = np.argsort(-(d0[rows_b] * (r0 > 0) + d1[rows_b] * (r1 > 0)))
        done = False
        for u in rows_b[pref][:24]:
            m0 = int(d0[u]) - r0
            m1 = int(d1[u]) - r1
            if m0 < 0 or m1 < 0:
                continue
            cand = ((d0 <= m0) & (d1 <= m1) & (blk != b)
                    & (L0[blk] + d0[u] - d0 <= caps[blk])
                    & (L1[blk] + d1[u] - d1 <= caps[blk]))
            idx = np.where(cand)[0]
            if len(idx) == 0:
                continue
            v = int(idx[np.argmax(d0[idx] + d1[idx])])
            b2 = int(blk[v])
            blk[u], blk[v] = b2, b
            L0[b] += d0[v] - d0[u]
            L1[b] += d1[v] - d1[u]
            L0[b2] += d0[u] - d0[v]
            L1[b2] += d1[u] - d1[v]
            done = True
            break
        if not done:
            stuck.add(b)

    # renumber blocks by tiles-needed desc, then load desc
    need = np.maximum((L0 + 127) >> 7, (L1 + 127) >> 7)
    loads = np.maximum(L0, L1)
    perm = np.lexsort((-loads, -need))       # heavy first
    rank = np.empty(OWNBLK, np.int64)
    rank[perm] = np.arange(OWNBLK)
    blk = rank[blk]
    slot = np.empty(n, np.int64)
    for b in range(OWNBLK):
        rows = np.where(blk == b)[0]
        slot[rows] = np.arange(len(rows))
    return blk, slot


def _glob_maps():
    """Local own-block j of parity h -> global pair block id, interleaving
    (even seg k, odd seg k); plus the er-table row-block for each global id."""
    glob = np.zeros((2, OWNBLK), np.int64)
    g = 0
    for k in range(NSEG):
        for h in (0, 1):
            for j in range(ECUM[k], ECUM[k + 1]):
                glob[h, j] = g
                g += 1
    er_of = np.zeros(NBLK, np.int64)
    for h in (0, 1):
        for j in range(OWNBLK):
            er_of[glob[h, j]] = h * OWNBLK + j
    return glob, er_of


def _tloc_of_slot(s):
    """own slot -> row within the (AG-part structured) half table slab
    layout; add ks*part_len inside."""
    if s < 3200:
        return 0, 3200, s
    if s < 5504:
        return 12800, 2304, s - 3200
    return 22016, 768, s - 5504


def _preprocess(src, dst):
    src = np.asarray(src).astype(np.int64)
    dst = np.asarray(dst).astype(np.int64)
    h_s = (src >= 25000).astype(np.int64)          # src half -> core parity

    dh = np.zeros((2, N), np.int64)
    np.add.at(dh[0], dst[h_s == 0], 1)
    np.add.at(dh[1], dst[h_s == 1], 1)

    # pack each (pair, dst-half) quarter strip into 49 balanced local blocks
    locblk = np.full(N, -1, np.int64)
    slot_of = np.full(N, -1, np.int64)
    for q in range(4):
        for h in range(2):
            lo = q * NPC if h == 0 else 25000 + q * NPC
            rows = np.arange(lo, lo + NPC)
            b, s = _pack(dh[0][rows], dh[1][rows])
            locblk[rows] = b
            slot_of[rows] = s

    glob, er_of = _glob_maps()

    node_q = np.where(np.arange(N) < 25000,
                      np.arange(N) // NPC, (np.arange(N) - 25000) // NPC)
    node_h = (np.arange(N) >= 25000).astype(np.int64)
    own_slot = locblk * 128 + slot_of
    node_core = 2 * node_q + node_h

    # table row of each node within its half table (AG-part layout)
    offs = np.empty(N, np.int64)
    plens = np.empty(N, np.int64)
    rels = np.empty(N, np.int64)
    s = own_slot
    m0 = s < 3200
    m1 = (s >= 3200) & (s < 5504)
    m2 = s >= 5504
    offs[m0], plens[m0], rels[m0] = 0, 3200, 0
    offs[m1], plens[m1], rels[m1] = 12800, 2304, 3200
    offs[m2], plens[m2], rels[m2] = 22016, 768, 5504
    tloc_n = offs + node_q * plens + (s - rels)

    own_nodes = np.full((NCORE, OWN), -1, np.int64)
    own_nodes[node_core, own_slot] = np.arange(N)

    # per-edge attributes (blocks in GLOBAL pair order)
    eb = glob[node_h[dst], locblk[dst]]
    es = slot_of[dst]
    et = tloc_n[src]
    ec = 2 * node_q[dst] + h_s

    counts = np.zeros((NCORE, NBLK), np.int64)
    for c in range(NCORE):
        counts[c] = np.bincount(eb[ec == c], minlength=NBLK)
    T_b = np.maximum(1, np.ceil(counts.max(axis=0) / 128).astype(np.int64))
    T = int(T_b.sum())
    off_b = np.zeros(NBLK + 1, np.int64)
    off_b[1:] = np.cumsum(T_b)

    per_core = []
    order = np.lexsort((eb, ec))
    s_core = ec[order]
    s_blk = eb[order]
    s_t = et[order]
    s_slot = es[order]
    for c in range(NCORE):
        sel = s_core == c
        cb = s_blk[sel]
        ct = s_t[sel]
        cs = s_slot[sel]
        idx_t = np.full(T * 128, DUMMY, np.int64)
        slots = np.zeros(T * 128, np.int64)
        start = np.searchsorted(cb, np.arange(NBLK))
        end = np.searchsorted(cb, np.arange(NBLK) + 1)
        for b in range(NBLK):
            nseg = end[b] - start[b]
            pos = int(off_b[b]) * 128
            idx_t[pos:pos + nseg] = ct[start[b]:end[b]]
            slots[pos:pos + nseg] = cs[start[b]:end[b]]
        per_core.append((idx_t, slots))

    tile_block = np.repeat(np.arange(NBLK), T_b)
    tile_first = np.zeros(T, bool)
    tile_last = np.zeros(T, bool)
    p = 0
    for b in range(NBLK):
        tile_first[p] = True
        tile_last[p + int(T_b[b]) - 1] = True
        p += int(T_b[b])
    # chunk index whose consumers finish each RS segment's partials
    seg_chunk = []
    for k in range(NSEG):
        last_tile = int(off_b[2 * ECUM[k + 1]]) - 1
        seg_chunk.append(last_tile // CHUNK)

    def wrap16(a):
        t = a.reshape(-1, 128)
        w = t.reshape(t.shape[0], 8, 16)
        return w.transpose(2, 0, 1).reshape(16, -1).astype(np.int16)

    cores = []
    for c in range(NCORE):
        idx_t, slots = per_core[c]
        cores.append(dict(
            idx16=wrap16(idx_t),
            slot=slots.reshape(T, 128).T.astype(ml_dtypes.bfloat16).copy(),
            slotf=slots.astype(ml_dtypes.bfloat16).reshape(1, T * 128),
        ))
    meta = dict(tile_block=tile_block, tile_first=tile_first,
                tile_last=tile_last, T=T, er_of=er_of, seg_chunk=seg_chunk)
    return cores, own_nodes, meta


def _augment(W, al, ar):
    dout = W.shape[1] // H
    Wal = np.stack([W[:, h * dout:(h + 1) * dout] @ al[h] for h in range(H)], 1)
    War = np.stack([W[:, h * dout:(h + 1) * dout] @ ar[h] for h in range(H)], 1)
    return np.concatenate([W, Wal, War], 1).astype(ml_dtypes.bfloat16)  # [128,136]


def _build(meta, consts, no_cc=False):
    import concourse.bass as bass
    import concourse.bacc as bacc
    import concourse.tile as tile
    from concourse import mybir
    from concourse.library_config import mlp

    tile_block = meta["tile_block"]
    tile_first = meta["tile_first"]
    tile_last = meta["tile_last"]
    T = meta["T"]
    er_of = meta["er_of"]
    seg_chunk = meta["seg_chunk"]

    f32 = mybir.dt.float32
    bf16 = mybir.dt.bfloat16
    i16 = mybir.dt.int16
    AF = mybir.ActivationFunctionType
    OP = mybir.AluOpType

    nc = bacc.Bacc(num_devices=NCORE)
    xT_in = nc.declare_dram_parameter("xT", [128, OWN], bf16, isOutput=False)
    idx16_in = nc.declare_dram_parameter("idx16", [16, T * 8], i16, isOutput=False)
    slot_in = nc.declare_dram_parameter("slotw", [128, T], bf16, isOutput=False)
    slotf_in = nc.declare_dram_parameter("slotf", [1, T * 128], bf16,
                                         isOutput=False)
    y_out = nc.declare_dram_parameter("y", [OWN, D], bf16, isOutput=True)

    chunks = []
    t0 = 0
    while t0 < T:
        chunks.append((t0, min(CHUNK, T - t0)))
        t0 += CHUNK

    groups_pair = [[2 * k, 2 * k + 1] for k in range(4)]
    groups_quad = [[0, 2, 4, 6], [1, 3, 5, 7]]

    with tile.TileContext(nc) as tc:
        with tc.tile_pool(name="persist", bufs=1) as pp, \
             tc.tile_pool(name="dram", bufs=1, space="DRAM") as dp:
            nc.gpsimd.load_library(mlp)

            # ---- persistent SBUF state ----
            idx_sb = pp.tile([128, T * 8], i16)
            for k in range(8):
                nc.sync.dma_start(out=idx_sb[k * 16:(k + 1) * 16, :],
                                  in_=idx16_in[:, :])
            slot_sb = pp.tile([128, T, 1], bf16)
            nc.sync.dma_start(out=slot_sb[:, :, 0], in_=slot_in[:, :])
            hT = pp.tile([128, OWN], bf16)
            nc.sync.dma_start(out=hT[:], in_=xT_in[:, :])
            hT2 = pp.tile([128, OWN], bf16)
            er_sb = [pp.tile([128, NBLK, 8], bf16, name=f"er_sb{i}")
                     for i in range(2)]

            iota_h = nc.inline_tensor(
                np.tile(np.arange(128).astype(ml_dtypes.bfloat16), (128, 1)),
                name="iota_row")
            iota_sb = pp.tile([128, 128], bf16)
            nc.sync.dma_start(out=iota_sb[:], in_=iota_h[:, :])
            iotap_h = nc.inline_tensor(
                np.arange(128).astype(ml_dtypes.bfloat16).reshape(128, 1),
                name="iota_part")
            iotap_sb = pp.tile([128, 1], bf16)
            nc.sync.dma_start(out=iotap_sb[:], in_=iotap_h[:, :])
            identb_h = nc.inline_tensor(np.eye(128, dtype=ml_dtypes.bfloat16),
                                        name="identb")
            identb_sb = pp.tile([128, 128], bf16)
            nc.sync.dma_start(out=identb_sb[:], in_=identb_h[:, :])

            waug_sb = []
            brep_sb = []
            for li in range(3):
                wh = nc.inline_tensor(consts[f"Waug{li}"], name=f"waug{li}")
                wt = pp.tile([128, 136], bf16, name=f"waug_sb{li}")
                nc.sync.dma_start(out=wt[:], in_=wh[:, :])
                waug_sb.append(wt)
                bh = nc.inline_tensor(consts[f"brep{li}"], name=f"brep{li}")
                bt = pp.tile([128, consts[f"brep{li}"].shape[1]], f32,
                             name=f"brep_sb{li}")
                nc.sync.dma_start(out=bt[:], in_=bh[:, :])
                brep_sb.append(bt)

            dummy_h = nc.inline_tensor(consts["dummyrow"], name="dummyrow")

            # ---- DRAM scratch (tables double-buffered) ----
            table = [dp.tile([TROWS, TCOLS], bf16, name=f"table{i}")
                     for i in range(2)]
            er_tab = [dp.tile([PAIR, 8], bf16, name=f"er_tab{i}")
                      for i in range(2)]
            ag_feat = [dp.tile([OWN, TCOLS], bf16, name=f"ag_feat{i}")
                       for i in range(2)]
            ag_er = [dp.tile([OWN, 8], bf16, name=f"ag_er{i}")
                     for i in range(2)]
            partial = dp.tile([PAIR, 132], f32)
            own_sum = dp.tile([OWN, 132], f32)

            for i in range(2):
                nc.sync.dma_start(out=table[i][DUMMY:DUMMY + 1, :],
                                  in_=dummy_h[:, :])

            def project(sp, ps, src_hT, li, j, tabrow, errow):
                """project own block j with layer li weights into table rows"""
                pj = ps.tile([128, 136], f32, space="PSUM", tag="pj")
                nc.tensor.matmul(pj[:], lhsT=src_hT[:, j * 128:(j + 1) * 128],
                                 rhs=waug_sb[li][:], start=True, stop=True)
                nc.vector.tensor_copy(tabrow[:, j, 0:132], pj[:, 0:132])
                nc.vector.tensor_tensor(
                    out=tabrow[:, j, 132:136], in0=pj[:, 128:132],
                    in1=tabrow[:, j, 128:132], op=OP.subtract)
                nc.scalar.activation(errow[:, j, 0:4], pj[:, 132:136], AF.Copy)
                nc.vector.tensor_tensor(
                    out=errow[:, j, 4:8], in0=pj[:, 132:136],
                    in1=errow[:, j, 0:4], op=OP.subtract)

            def flush_tab(buf, tabrow, errow, j0, j1):
                """write projected blocks [j0,j1) to the AG staging buffers"""
                nc.sync.dma_start(
                    out=ag_feat[buf][j0 * 128:j1 * 128, :]
                        .rearrange("(t p) c -> p t c", p=128),
                    in_=tabrow[:, j0:j1, :])
                nc.sync.dma_start(
                    out=ag_er[buf][j0 * 128:j1 * 128, :]
                        .rearrange("(t p) c -> p t c", p=128),
                    in_=errow[:, j0:j1, :])

            def ag_part(buf, p):
                j0, j1 = AGPART[p]
                plen = (j1 - j0) * 128
                if no_cc:
                    for rep in range(4):
                        nc.sync.dma_start(
                            out=table[buf][AGOFF[p] + rep * plen:
                                           AGOFF[p] + (rep + 1) * plen, :],
                            in_=ag_feat[buf][j0 * 128:j1 * 128, :])
                else:
                    nc.gpsimd.collective_compute(
                        "AllGather", mybir.AluOpType.bypass,
                        replica_groups=groups_quad,
                        ins=[ag_feat[buf][j0 * 128:j1 * 128, :]],
                        outs=[table[buf][AGOFF[p]:AGOFF[p] + 4 * plen, :]])

            def ag_er_all(buf):
                if no_cc:
                    for rep in range(2):
                        nc.sync.dma_start(
                            out=er_tab[buf][rep * OWN:(rep + 1) * OWN, :],
                            in_=ag_er[buf][:, :])
                else:
                    nc.gpsimd.collective_compute(
                        "AllGather", mybir.AluOpType.bypass,
                        replica_groups=groups_pair,
                        ins=[ag_er[buf][:, :]], outs=[er_tab[buf][:, :]])
                nc.sync.dma_start(
                    out=er_sb[buf][:],
                    in_=er_tab[buf][:, :].rearrange("(b p) c -> p b c", p=128))

            # ---- layer-0 projection + table build ----
            with tc.tile_pool(name="prj0", bufs=2) as sp, \
                 tc.tile_pool(name="prjps0", bufs=2, space="PSUM") as ps:
                tabrow = sp.tile([128, OWNBLK, TCOLS], bf16, name="tabrow0",
                                 tag="tabrow", bufs=1)
                errow = sp.tile([128, OWNBLK, 8], bf16, name="errow0",
                                tag="errow", bufs=1)
                for j in range(OWNBLK):
                    project(sp, ps, hT, 0, j, tabrow, errow)
                    for p, (j0, j1) in enumerate(AGPART):
                        if j1 == j + 1:
                            flush_tab(0, tabrow, errow, j0, j1)
                            ag_part(0, p)
                ag_er_all(0)

            for li in range(3):
                buf = li % 2
                nbuf = (li + 1) % 2
                src_hT = hT if li % 2 == 0 else hT2
                dst_hT = hT2 if li % 2 == 0 else hT
                last = li == 2

                with tc.tile_pool(name=f"gt{li}", bufs=3) as gp, \
                     tc.tile_pool(name=f"sf{li}", bufs=2) as sfp, \
                     tc.tile_pool(name=f"ms{li}", bufs=3) as mp, \
                     tc.tile_pool(name=f"sm{li}", bufs=2) as smp, \
                     tc.tile_pool(name=f"smt{li}", bufs=2) as smtp, \
                     tc.tile_pool(name=f"ex{li}", bufs=4) as xp, \
                     tc.tile_pool(name=f"pb{li}", bufs=4) as pbp, \
                     tc.tile_pool(name=f"po{li}", bufs=4) as pop, \
                     tc.tile_pool(name=f"os{li}", bufs=2) as osp, \
                     tc.tile_pool(name=f"pr{li}", bufs=2) as prp, \
                     tc.tile_pool(name=f"erp{li}", bufs=2, space="PSUM") as erps, \
                     tc.tile_pool(name=f"sg{li}", bufs=3, space="PSUM") as sgps, \
                     tc.tile_pool(name=f"pj{li}", bufs=1, space="PSUM") as pjps, \
                     tc.tile_pool(name=f"tp{li}", bufs=2, space="PSUM") as tps:

                    if not last:
                        tabrow = prp.tile([128, OWNBLK, TCOLS], bf16,
                                          name=f"tabrow{li + 1}", tag="tabrow",
                                          bufs=1)
                        errow = prp.tile([128, OWNBLK, 8], bf16,
                                         name=f"errow{li + 1}", tag="errow",
                                         bufs=1)

                    def emit_segment(k):
                        j0, j1 = ECUM[k], ECUM[k + 1]
                        if no_cc:
                            nc.sync.dma_start(
                                out=own_sum[j0 * 128:j1 * 128, :],
                                in_=partial[2 * j0 * 128:
                                            (2 * j0 + (j1 - j0)) * 128, :])
                        else:
                            nc.gpsimd.collective_compute(
                                "ReduceScatter", mybir.AluOpType.add,
                                replica_groups=groups_pair,
                                ins=[partial[2 * j0 * 128:2 * j1 * 128, :]],
                                outs=[own_sum[j0 * 128:j1 * 128, :]])
                        osum = osp.tile([128, 7, 132], f32, tag="osum")
                        nc.sync.dma_start(
                            out=osum[:, 0:j1 - j0, :],
                            in_=own_sum[j0 * 128:j1 * 128, :]
                                .rearrange("(t p) c -> p t c", p=128))
                        for j in range(j0, j1):
                            jj = j - j0
                            den = pop.tile([128, 4], f32, tag="den")
                            nc.vector.tensor_scalar_max(
                                den[:], osum[:, jj, 128:132], EPS)
                            rec = pop.tile([128, 4, 1], f32, tag="rec")
                            nc.vector.reciprocal(rec[:, :, 0], den[:])
                            o = pop.tile([128, 4, 32], f32, tag="o")
                            nc.vector.tensor_tensor(
                                out=o[:], in0=osum[:, jj, 0:128],
                                in1=rec[:].to_broadcast([128, 4, 32]),
                                op=OP.mult)
                            if not last:
                                o2 = pop.tile([128, 128], f32, tag="o2")
                                nc.vector.tensor_tensor(
                                    out=o2[:], in0=o[:], in1=brep_sb[li][:],
                                    op=OP.add)
                                # ELU: max(x,0) + exp(min(x,0)) - 1
                                mn = pop.tile([128, 128], f32, tag="mn")
                                nc.vector.tensor_scalar_min(mn[:], o2[:], 0.0)
                                exn = pop.tile([128, 128], f32, tag="exn")
                                nc.scalar.activation(exn[:], mn[:], AF.Exp)
                                nc.vector.tensor_scalar_max(o2[:], o2[:], 0.0)
                                nc.vector.tensor_tensor(out=o2[:], in0=o2[:],
                                                        in1=exn[:], op=OP.add)
                                o2b = pop.tile([128, 128], bf16, tag="o2b")
                                nc.vector.tensor_scalar_add(o2b[:], o2[:], -1.0)
                                tp = tps.tile([128, 128], bf16, space="PSUM",
                                              tag="tp")
                                nc.tensor.transpose(tp[:], o2b[:],
                                                    identb_sb[:])
                                nc.scalar.activation(
                                    dst_hT[:, j * 128:(j + 1) * 128], tp[:],
                                    AF.Copy)
                                project(prp, pjps, dst_hT, li + 1, j,
                                        tabrow, errow)
                            else:
                                r1 = pop.tile([128, 32], f32, tag="r1")
                                nc.vector.tensor_tensor(
                                    out=r1[:], in0=o[:, 0, :], in1=o[:, 1, :],
                                    op=OP.add)
                                r2 = pop.tile([128, 32], f32, tag="r2")
                                nc.vector.tensor_tensor(
                                    out=r2[:], in0=o[:, 2, :], in1=o[:, 3, :],
                                    op=OP.add)
                                nc.vector.tensor_tensor(
                                    out=r1[:], in0=r1[:], in1=r2[:], op=OP.add)
                                nc.vector.tensor_scalar_mul(r1[:], r1[:], 0.25)
                                r1b = pop.tile([128, 32], bf16, tag="r1b")
                                nc.vector.tensor_tensor(
                                    out=r1b[:], in0=r1[:],
                                    in1=brep_sb[li][:, 0:32], op=OP.add)
                                nc.sync.dma_start(
                                    out=y_out[j * 128:(j + 1) * 128, :],
                                    in_=r1b[:])
                        if not last:
                            for p, (a0, a1) in enumerate(AGPART):
                                if a1 == j1:
                                    flush_tab(nbuf, tabrow, errow, a0, a1)
                                    ag_part(nbuf, p)
                        if k == NSEG - 1 and not last:
                            ag_er_all(nbuf)

                    # ---- edge phase (segment work interleaved) ----
                    seg = None
                    next_seg = [0]
                    for ci, (c0, clen) in enumerate(chunks):
                        g = gp.tile([128, CHUNK, TCOLS], bf16, tag="g")
                        nc.gpsimd.dma_gather(
                            out_ap=g[:, 0:clen, :], in_ap=table[buf][:, :],
                            idxs_ap=idx_sb[:, c0 * 8:(c0 + clen) * 8],
                            num_idxs=clen * 128, num_idxs_reg=clen * 128,
                            elem_size=TCOLS, single_packet=False)
                        sfr = sfp.tile([128, CHUNK, 128], bf16, tag="sfr")
                        nc.sync.dma_start(
                            out=sfr[:, 0:clen, :],
                            in_=slotf_in[0:1, c0 * 128:(c0 + clen) * 128]
                                .rearrange("o (t i) -> o t i", i=128)
                                .to_broadcast([128, clen, 128]))
                        for g0 in range(0, clen, GROUP):
                            gl = min(GROUP, clen - g0)
                            smat = smp.tile([128, GROUP, 128], bf16, tag="smat")
                            nc.vector.tensor_tensor(
                                out=smat[:, 0:gl, :],
                                in0=slot_sb[:, c0 + g0:c0 + g0 + gl, :]
                                    .to_broadcast([128, gl, 128]),
                                in1=iota_sb[:].rearrange("p (t i) -> p t i", t=1)
                                    .to_broadcast([128, gl, 128]),
                                op=OP.is_equal)
                            smatT = smtp.tile([128, GROUP, 128], bf16,
                                              tag="smatT")
                            nc.vector.tensor_tensor(
                                out=smatT[:, 0:gl, :],
                                in0=sfr[:, g0:g0 + gl, :],
                                in1=iotap_sb[:, :]
                                    .rearrange("p (t i) -> p t i", t=1)
                                    .to_broadcast([128, gl, 128]),
                                op=OP.is_equal)
                            erp = erps.tile([128, GROUP, 8], f32, space="PSUM",
                                            tag="erp")
                            for t in range(gl):
                                gt = c0 + g0 + t
                                b = int(tile_block[gt])
                                nc.tensor.matmul(
                                    erp[:, t, :], lhsT=smatT[:, t, :],
                                    rhs=er_sb[buf][:, int(er_of[b]), :],
                                    start=True, stop=True)
                            e4 = xp.tile([128, GROUP, 4], f32, tag="e4")
                            nc.vector.tensor_tensor(
                                out=e4[:, 0:gl, :], in0=g[:, g0:g0 + gl, 128:132],
                                in1=g[:, g0:g0 + gl, 132:136], op=OP.add)
                            nc.vector.tensor_tensor(
                                out=e4[:, 0:gl, :], in0=erp[:, 0:gl, 0:4],
                                in1=e4[:, 0:gl, :], op=OP.add)
                            nc.vector.tensor_tensor(
                                out=e4[:, 0:gl, :], in0=erp[:, 0:gl, 4:8],
                                in1=e4[:, 0:gl, :], op=OP.add)
                            t4 = xp.tile([128, GROUP, 4], f32, tag="t4")
                            nc.scalar.activation(t4[:, 0:gl, :], e4[:, 0:gl, :],
                                                 AF.Copy, scale=NEG)
                            nc.vector.tensor_tensor(
                                out=e4[:, 0:gl, :], in0=e4[:, 0:gl, :],
                                in1=t4[:, 0:gl, :], op=OP.max)
                            ex4 = xp.tile([128, GROUP, 4, 1], f32, tag="ex4")
                            nc.scalar.activation(ex4[:, 0:gl, :, 0],
                                                 e4[:, 0:gl, :], AF.Exp)
                            m4 = mp.tile([128, GROUP, 132], bf16, tag="m4")
                            nc.scalar.activation(m4[:, 0:gl, 128:132],
                                                 ex4[:, 0:gl, :, 0], AF.Copy)
                            nc.vector.tensor_tensor(
                                out=m4[:, 0:gl, 0:128],
                                in0=g[:, g0:g0 + gl, 0:128],
                                in1=ex4[:, 0:gl, :, :]
                                    .to_broadcast([128, gl, 4, 32]),
                                op=OP.mult)
                            for t in range(gl):
                                gt = c0 + g0 + t
                                b = int(tile_block[gt])
                                if tile_first[gt]:
                                    seg = sgps.tile([128, 132], f32,
                                                    space="PSUM", tag="seg",
                                                    name=f"seg{li}_{b}")
                                nc.tensor.matmul(
                                    seg[:], lhsT=smat[:, t, :], rhs=m4[:, t, :],
                                    start=bool(tile_first[gt]),
                                    stop=bool(tile_last[gt]))
                                if tile_last[gt]:
                                    pb = pbp.tile([128, 132], f32, tag="pb")
                                    nc.scalar.activation(pb[:], seg[:], AF.Copy)
                                    nc.sync.dma_start(
                                        out=partial[b * 128:(b + 1) * 128, :],
                                        in_=pb[:])
                        while (next_seg[0] < NSEG
                               and seg_chunk[next_seg[0]] <= ci):
                            emit_segment(next_seg[0])
                            next_seg[0] += 1
                    while next_seg[0] < NSEG:
                        emit_segment(next_seg[0])
                        next_seg[0] += 1
    nc.finalize()
    return nc


def _consts(W0, al0, ar0, b0, W1, al1, ar1, b1, W2, al2, ar2, b2):
    consts = {}
    for li, (W, al, ar, b) in enumerate(
            [(W0, al0, ar0, b0), (W1, al1, ar1, b1), (W2, al2, ar2, b2)]):
        consts[f"Waug{li}"] = _augment(np.asarray(W, np.float32),
                                       np.asarray(al, np.float32),
                                       np.asarray(ar, np.float32))
        b = np.asarray(b, np.float32)
        if li < 2:
            consts[f"brep{li}"] = np.tile(b.reshape(1, 128), (128, 1))
        else:
            consts[f"brep{li}"] = np.tile(b.reshape(H, D).mean(0).reshape(1, D),
                                          (128, 1))
    dummy = np.zeros((1, TCOLS), ml_dtypes.bfloat16)
    dummy[0, 128:132] = ml_dtypes.bfloat16(-1e30)
    consts["dummyrow"] = dummy
    return consts


def _graph_state(src, dst):
    key = (hash(np.asarray(src).tobytes()) ^ hash(np.asarray(dst).tobytes()))
    if _cache.get("prekey") != key:
        _cache["pre"] = _preprocess(src, dst)
        _cache["prekey"] = key
        _cache.pop("nckey", None)
        _cache.pop("xkey", None)
    return _cache["pre"]


def _in_maps(x, own_nodes, cores):
    x = np.asarray(x, np.float32)
    maps = []
    for c in range(NCORE):
        vn = own_nodes[c]
        xT = np.zeros((128, OWN), ml_dtypes.bfloat16)
        valid = vn >= 0
        xT[:, valid] = x[vn[valid]].T.astype(ml_dtypes.bfloat16)
        maps.append(dict(xT=xT, idx16=cores[c]["idx16"],
                         slotw=np.asarray(cores[c]["slot"]),
                         slotf=np.asarray(cores[c]["slotf"])))
    return maps


def _run_cached(nc, in_maps):
    """Multi-core PJRT execution with device-resident input caching."""
    import jax
    import jax.numpy as jnp
    from jax.sharding import Mesh, PartitionSpec, NamedSharding
    try:
        from jax.experimental.shard_map import shard_map
    except ImportError:
        from jax.sharding import shard_map
    from concourse import bass2jax
    from concourse import mybir

    st = _cache.get("run")
    if st is None or st["nc"] is not nc:
        bass2jax.install_neuronx_cc_hook()
        partition_name = (nc.partition_id_tensor.name
                          if nc.partition_id_tensor else None)
        in_names, out_names, out_avals = [], [], []
        for alloc in nc.m.functions[0].allocations:
            if not isinstance(alloc, mybir.MemoryLocationSet):
                continue
            name = alloc.memorylocations[0].name
            if alloc.kind == "ExternalInput":
                if name != partition_name:
                    in_names.append(name)
            elif alloc.kind == "ExternalOutput":
                out_names.append(name)
                out_avals.append(jax.core.ShapedArray(
                    tuple(alloc.tensor_shape), mybir.dt.np(alloc.dtype)))
        n_params = len(in_names)
        n_outs = len(out_avals)
        all_in_names = list(in_names) + list(out_names)
        if partition_name is not None:
            all_in_names.append(partition_name)
        donate = tuple(range(n_params, n_params + n_outs))

        def _body(*args):
            operands = list(args)
            if partition_name is not None:
                operands.append(bass2jax.partition_id_tensor())
            outs = bass2jax._bass_exec_p.bind(
                *operands,
                out_avals=tuple(out_avals),
                in_names=tuple(all_in_names),
                out_names=tuple(out_names),
                lowering_input_output_aliases=(),
                sim_require_finite=True,
                sim_require_nnan=True,
                nc=nc,
            )
            return tuple(outs)

        devices = jax.devices()[:NCORE]
        mesh = Mesh(np.asarray(devices), ("core",))
        in_specs = (PartitionSpec("core"),) * (n_params + n_outs)
        out_specs = (PartitionSpec("core"),) * n_outs
        fn = jax.jit(
            shard_map(_body, mesh=mesh, in_specs=in_specs,
                      out_specs=out_specs, check_rep=False),
            donate_argnums=donate, keep_unused=True)
        st = dict(nc=nc, fn=fn, mesh=mesh, in_names=in_names,
                  out_names=out_names, out_avals=out_avals,
                  dev_in={}, dev_key={})
        _cache["run"] = st

    sharding = NamedSharding(st["mesh"], PartitionSpec("core"))
    dev_args = []
    for name in st["in_names"]:
        concat = np.concatenate([np.asarray(m[name]) for m in in_maps], axis=0)
        key = hash(concat.tobytes())
        if st["dev_key"].get(name) != key:
            st["dev_in"][name] = jax.device_put(concat, sharding)
            st["dev_key"][name] = key
        dev_args.append(st["dev_in"][name])
    zeros = []
    for av in st["out_avals"]:
        shp = (NCORE * av.shape[0], *av.shape[1:])
        try:
            z = jnp.zeros(shp, av.dtype, device=sharding)
        except TypeError:
            z = jax.device_put(np.zeros(shp, av.dtype), sharding)
        zeros.append(z)
    outs = st["fn"](*dev_args, *zeros)
    return {name: np.asarray(outs[i]).reshape(NCORE, *st["out_avals"][i].shape)
            for i, name in enumerate(st["out_names"])}


def kernel(x, src, dst, W0, al0, ar0, b0, W1, al1, ar1, b1, W2, al2, ar2, b2):
    cores, own_nodes, meta = _graph_state(src, dst)

    consts = _consts(W0, al0, ar0, b0, W1, al1, ar1, b1, W2, al2, ar2, b2)
    ck = _cache["prekey"] ^ hash(consts["Waug0"].tobytes()) \
        ^ hash(consts["Waug1"].tobytes()) ^ hash(consts["Waug2"].tobytes())
    if _cache.get("nckey") != ck:
        _cache["nc"] = _build(meta, consts)
        _cache["nckey"] = ck
        _cache.pop("run", None)
    nc = _cache["nc"]

    x = np.asarray(x, np.float32)
    xk = hash(x.tobytes())
    if _cache.get("xkey") != xk:
        _cache["maps"] = _in_maps(x, own_nodes, cores)
        _cache["xkey"] = xk
    maps = _cache["maps"]

    outs = _run_cached(nc, maps)
    y = np.zeros((N, D), np.float32)
    for c in range(NCORE):
        vn = own_nodes[c]
        valid = vn >= 0
        y[vn[valid]] = outs["y"][c][valid].astype(np.float32)
    return y
